# revision 2
# baseline (speedup 1.0000x reference)
"""Trainium2 Bass kernel for nn_CrossAttention (B=2, I=J=2048, E=1024, H=16, D=64).

Sharding: 8 cores = data parallel on batch (2) x tensor parallel on heads
(4 groups of 4 heads).  Core c handles batch c//4, heads 4*(c%4) .. 4*(c%4)+3.
Each core computes a partial output projection (its heads' slice of Wo rows);
the host sums the 4 partials per batch and adds bo.

Device-side dataflow (transposed layout; contraction dim on partitions):
  qT = Wq_g^T @ query^T          [256, 2048]   (Wq pre-scaled by D**-0.5)
  kT = Wk_g^T @ key^T            [256, 2048]
  v  = value @ Wv_g (+ones col)  [2048, 4*65]
  per head PAIR (2e, 2e+1), per i-half (1024 cols), software-pipelined jc loop:
    simT_h[j,i] = matmul(lhsT=kT[e][64h%2 slice], rhs=qT[e][same slice])
                  -- K=64 row-tiled: both heads run CONCURRENTLY in the array
    PT_h = exp(simT_h) * exp(biasT_h)   (ACT exp, DVE mult, bf16)
    o_ps_h[65, 512] += [v_h|1]^T @ PT_h  (row 64 = softmax denominator)
  normalize via transpose + reciprocal (as before), merge heads -> om
  yT = Wo_g^T' @ outT   [1024, 2048] bf16 -> DRAM
Host: out[b] = sum_g yT_g^T + bo  (f32 accumulation of bf16 partials).
"""

import os
import numpy as np
import ml_dtypes

import concourse.bass as bass
import concourse.tile as tile
from concourse import bacc, mybir
from concourse.bass_utils import run_bass_kernel_spmd
from concourse.masks import make_identity

BF16 = ml_dtypes.bfloat16
F32 = mybir.dt.float32
BF = mybir.dt.bfloat16

B, I, J = 2, 2048, 2048
E, H = 1024, 16
QE, KE = 1024, 1024
D = E // H                      # 64
SCALE = D ** -0.5
NEG = -1e20

N_CORES = 8
HPC = H // 4                    # 4 heads per core
EC = HPC * D                    # 256 E-columns per core
P = 128

# module-level switches (test.py pokes these)
PROFILE = bool(os.environ.get("KERNEL_PROFILE"))
LAST_EXEC_TIME_NS = None

_CACHED = None  # compiled Bass module
_HOOK_READY = False


def _ensure_profile_hooks():
    """Dev-only: register the NTFF profile hook that the agent image's
    antenv package lacks, and stub out the artifact upload (no bucket
    creds here).  Only used when PROFILE is on; the plain execution
    path never touches any of this."""
    global _HOOK_READY
    if _HOOK_READY:
        return
    import contextlib
    import ctypes
    import sys
    import types

    from concourse import bass_utils as bu

    bu.upload_artifacts = lambda tmpdir: "local://" + tmpdir

    try:
        from antenv.axon_hooks import get_axon_ntff_profile_hook  # noqa: F401
        _HOOK_READY = True
        return
    except ImportError:
        pass

    so_path = "/opt/axon/libaxon_pjrt.so"
    lib = ctypes.CDLL(so_path)
    assert hasattr(lib, "axon_start_nrt_profile"), "old libaxon_pjrt.so"
    lib.axon_start_nrt_profile.argtypes = [
        ctypes.POINTER(ctypes.c_int64), ctypes.c_size_t]
    lib.axon_start_nrt_profile.restype = ctypes.c_int64
    lib.axon_stop_nrt_profile.argtypes = [ctypes.c_char_p]
    lib.axon_stop_nrt_profile.restype = ctypes.c_int64

    @contextlib.contextmanager
    def _hook(output_dir, device_ids):
        import jax
        jax.devices()
        if device_ids:
            ids = (ctypes.c_int64 * len(device_ids))(*device_ids)
            rc = lib.axon_start_nrt_profile(ids, len(device_ids))
        else:
            rc = lib.axon_start_nrt_profile(None, 0)
        if rc != 0:
            raise RuntimeError(f"axon_start_nrt_profile rc={rc}")
        try:
            yield
        finally:
            n = lib.axon_stop_nrt_profile(str(output_dir).encode())
            if n < 0:
                raise RuntimeError(f"axon_stop_nrt_profile rc={n}")

    mod = types.ModuleType("antenv.axon_hooks")
    mod.get_axon_ntff_profile_hook = lambda: _hook
    mod.set_axon_ntff_profile_hook = lambda h: None
    sys.modules["antenv.axon_hooks"] = mod
    _HOOK_READY = True


def _build_program():
    nc = bacc.Bacc("TRN2", debug=False, enable_asserts=False,
                   target_bir_lowering=False, num_devices=N_CORES)

    qt_d = nc.dram_tensor("qt", [QE, I], BF, kind="ExternalInput").ap()
    kt_d = nc.dram_tensor("kt", [KE, J], BF, kind="ExternalInput").ap()
    vt_d = nc.dram_tensor("vt", [KE, J], BF, kind="ExternalInput").ap()
    wq_d = nc.dram_tensor("wq", [QE, EC], BF, kind="ExternalInput").ap()
    wk_d = nc.dram_tensor("wk", [KE, EC], BF, kind="ExternalInput").ap()
    wv_d = nc.dram_tensor("wv", [KE, EC], BF, kind="ExternalInput").ap()
    wo_d = nc.dram_tensor("wo", [EC, QE], BF, kind="ExternalInput").ap()
    bias_d = nc.dram_tensor("bias_t", [HPC, J, I], BF, kind="ExternalInput").ap()
    yt_d = nc.dram_tensor("yt", [QE, I], BF, kind="ExternalOutput").ap()

    KC = QE // P                # 8 contraction chunks for the projections
    ICN = I // 512              # 4 free-dim chunks of 512
    JC = J // P                 # 16 key chunks of 128
    VW = D + 1                  # 65: per-head v columns + ones column
    IHW = 1024                  # i-half width
    NIH = I // IHW              # 2 i-halves

    with tile.TileContext(nc) as tc:
        with (
            tc.tile_pool(name="w", bufs=1) as w_pool,
            tc.tile_pool(name="io", bufs=8) as io_pool,
            tc.tile_pool(name="persist", bufs=1) as pp,
            tc.tile_pool(name="bias", bufs=6) as bias_pool,
            tc.tile_pool(name="x", bufs=6) as x_pool,
            tc.tile_pool(name="pt", bufs=6) as pt_pool,
            tc.tile_pool(name="osb", bufs=8) as osb_pool,
            tc.tile_pool(name="rcp", bufs=4) as rcp_pool,
            tc.tile_pool(name="y", bufs=3) as y_pool,
            tc.tile_pool(name="pssim", bufs=2, space="PSUM") as ps_sim,
            tc.tile_pool(name="psot", bufs=4, space="PSUM") as ps_ot,
        ):
            # ---- weights ----
            wq_s = w_pool.tile([P, KC, EC], BF)
            wk_s = w_pool.tile([P, KC, EC], BF)
            wv_s = w_pool.tile([P, KC, EC], BF)
            wo_s = w_pool.tile([P, 2, QE], BF)
            nc.sync.dma_start(wq_s, wq_d.rearrange("(kc p) n -> p kc n", p=P))
            nc.sync.dma_start(wk_s, wk_d.rearrange("(kc p) n -> p kc n", p=P))
            nc.sync.dma_start(wv_s, wv_d.rearrange("(kc p) n -> p kc n", p=P))
            nc.sync.dma_start(wo_s, wo_d.rearrange("(ec p) n -> p ec n", p=P))
            ident = w_pool.tile([P, P], BF)
            make_identity(nc, ident)

            # ---- projections ----
            # kT_s / qT_s: [2 x (128 E-rows), N]; head h lives in tile h//2,
            # partitions 64*(h%2) .. +64.  The K=64 slices feed row-tiled
            # sim matmuls (auto tile_position from base partition).
            kT_s = [pp.tile([P, J], BF, name=f"kT{e}") for e in range(2)]
            qT_s = [pp.tile([P, I], BF, name=f"qT{e}") for e in range(2)]
            # v natural layout with a ones column per head: [J, 4*65]
            v_s = pp.tile([P, JC, HPC * VW], BF, name="v_s")
            for h in range(HPC):
                nc.gpsimd.memset(v_s[:, :, h * VW + D : h * VW + D + 1], 1.0)

            def load_chunks(src):
                ch = []
                for kc in range(KC):
                    t = io_pool.tile([P, I], BF, tag="io")
                    nc.sync.dma_start(t, src[kc * P : (kc + 1) * P, :])
                    ch.append(t)
                return ch

            # q/k projections -> packed transposed layout
            for src, dst in ((qt_d, qT_s), (kt_d, kT_s)):
                chunks = load_chunks(src)
                for e in range(2):
                    for ic in range(ICN):
                        isl = slice(ic * 512, (ic + 1) * 512)
                        ps = ps_ot.tile([P, 512], F32, tag="ot", name="p_ps")
                        for kc in range(KC):
                            nc.tensor.matmul(
                                ps,
                                lhsT=(wq_s if src is qt_d else wk_s)[
                                    :, kc, e * P : (e + 1) * P],
                                rhs=chunks[kc][:, isl],
                                start=(kc == 0), stop=(kc == KC - 1),
                            )
                        nc.vector.tensor_copy(dst[e][:, isl], ps)

            # v projection -> natural layout [J, EC], head-strided with ones col
            chunks = load_chunks(vt_d)
            for jc in range(JC):
                ps = ps_ot.tile([P, EC], F32, tag="ot", name="v_ps")
                for kc in range(KC):
                    nc.tensor.matmul(
                        ps,
                        lhsT=chunks[kc][:, jc * P : (jc + 1) * P],
                        rhs=wv_s[:, kc, :],
                        start=(kc == 0), stop=(kc == KC - 1),
                    )
                for h in range(HPC):
                    nc.vector.tensor_copy(
                        v_s[:, jc, h * VW : h * VW + D],
                        ps[:, h * D : (h + 1) * D],
                    )

            # out_merged: normalized attention output in natural layout
            om = pp.tile([P, I // P, EC], BF, name="om")

            # ---- attention: head pairs, i-halves, pipelined jc loop ----
            for e in range(2):          # head pair (2e, 2e+1)
                osbs = {0: [], 1: []}   # per local head: 4 i-chunk tiles
                for ih in range(NIH):
                    i0 = ih * IHW
                    o_ps = [[ps_ot.tile([VW, 512], F32, tag="ot",
                                        name=f"o{e}_{ih}_{hh}_{c}")
                             for c in range(2)] for hh in range(2)]
                    pts = [None, None]  # previous step's pt pair
                    for jc in range(JC + 1):
                        if jc < JC:
                            jsl = slice(jc * P, (jc + 1) * P)
                            new_pts = []
                            for hh in range(2):
                                h = 2 * e + hh
                                psl = slice(hh * D, (hh + 1) * D)
                                eb = bias_pool.tile([P, IHW], BF, tag="bias")
                                nc.sync.dma_start(
                                    eb, bias_d[h, jsl, i0 : i0 + IHW])
                                s_ps = ps_sim.tile([P, IHW], F32, tag="sim")
                                for c in range(2):
                                    nc.tensor.matmul(
                                        s_ps[:, c * 512 : (c + 1) * 512],
                                        lhsT=kT_s[e][psl, jsl],
                                        rhs=qT_s[e][psl,
                                                    i0 + c * 512 : i0 + (c + 1) * 512],
                                        start=True, stop=True,
                                    )
                                x_t = x_pool.tile([P, IHW], BF, tag="x")
                                nc.scalar.activation(
                                    x_t, s_ps, mybir.ActivationFunctionType.Exp)
                                pt = pt_pool.tile([P, IHW], BF, tag="pt")
                                nc.vector.tensor_tensor(
                                    pt, x_t, eb, mybir.AluOpType.mult)
                                new_pts.append(pt)
                        if jc >= 1:
                            pjc = jc - 1
                            for hh in range(2):
                                h = 2 * e + hh
                                for c in range(2):
                                    nc.tensor.matmul(
                                        o_ps[hh][c],
                                        lhsT=v_s[:, pjc, h * VW : (h + 1) * VW],
                                        rhs=pts[hh][:, c * 512 : (c + 1) * 512],
                                        start=(pjc == 0), stop=(pjc == JC - 1),
                                    )
                        if jc < JC:
                            pts = new_pts
                    for hh in range(2):
                        for c in range(2):
                            osb = osb_pool.tile([VW, 512], BF, tag="osb",
                                                name=f"osb{e}_{ih}_{hh}_{c}")
                            nc.vector.tensor_copy(osb, o_ps[hh][c])
                            osbs[hh].append(osb)

                # normalize in natural layout: transpose [65, 128] blocks to
                # [128, 65]; recip of col 64 is then a per-partition scalar
                for hh in range(2):
                    h = 2 * e + hh
                    for ic in range(ICN):
                        osb = osbs[hh][ic]
                        nat = ps_ot.tile([P, ICN, VW + 1], BF, tag="ot",
                                         name=f"nat{h}_{ic}")
                        for t in range(ICN):
                            nc.tensor.transpose(
                                nat[:, t, 0:VW],
                                osb[:, t * P : (t + 1) * P],
                                ident[0:VW, 0:VW],
                            )
                        for t in range(ICN):
                            rcp = rcp_pool.tile([P, 1], F32, tag="rcp")
                            nc.vector.reciprocal(rcp, nat[:, t, D : D + 1])
                            nc.vector.tensor_scalar_mul(
                                om[:, ICN * ic + t, h * D : (h + 1) * D],
                                nat[:, t, 0:D], rcp,
                            )

            # ---- transpose out_merged back to [EC, I] for the final matmul
            outT = [pp.tile([P, I], BF, name=f"outT{e}") for e in range(2)]
            for i16 in range(I // P):
                for e in range(2):
                    t2 = ps_ot.tile([P, P], BF, tag="ot", name=f"t2_{i16}_{e}")
                    nc.tensor.transpose(
                        t2, om[:, i16, e * P : (e + 1) * P], ident)
                    nc.scalar.copy(outT[e][:, i16 * P : (i16 + 1) * P], t2)

            # ---- output projection: yT = Wo_g^T-contraction over EC ----
            for oc in range(QE // P):
                for ic in range(ICN):
                    ps = ps_ot.tile([P, 512], F32, tag="ot", name="y_ps")
                    for e in range(2):
                        nc.tensor.matmul(
                            ps,
                            lhsT=wo_s[:, e, oc * P : (oc + 1) * P],
                            rhs=outT[e][:, ic * 512 : (ic + 1) * 512],
                            start=(e == 0), stop=(e == 1),
                        )
                    y_sb = y_pool.tile([P, 512], BF, tag="y")
                    nc.vector.tensor_copy(y_sb, ps)
                    nc.sync.dma_start(
                        yt_d[oc * P : (oc + 1) * P, ic * 512 : (ic + 1) * 512],
                        y_sb,
                    )

    nc.compile()
    return nc


def kernel(query, key, value, query_mask, key_mask, rel_pos_bias,
           Wq, Wk, Wv, Wo, bo):
    global _CACHED, LAST_EXEC_TIME_NS
    query = np.asarray(query, dtype=np.float32)
    key = np.asarray(key, dtype=np.float32)
    value = np.asarray(value, dtype=np.float32)
    rel_pos_bias = np.asarray(rel_pos_bias, dtype=np.float32)
    query_mask = np.asarray(query_mask)
    key_mask = np.asarray(key_mask)
    Wq, Wk, Wv = (np.asarray(w, dtype=np.float32) for w in (Wq, Wk, Wv))
    Wo = np.asarray(Wo, dtype=np.float32)
    bo = np.asarray(bo, dtype=np.float32)

    if not key_mask.all():
        rel_pos_bias = rel_pos_bias + np.where(key_mask, 0.0, NEG)[:, None, None, :]

    if _CACHED is None:
        _CACHED = _build_program()
    nc = _CACHED

    # per-batch transposed activations (shared by the 4 cores of that batch)
    qT = [np.ascontiguousarray(query[b].T).astype(BF16) for b in range(B)]
    kT = [np.ascontiguousarray(key[b].T).astype(BF16) for b in range(B)]
    vT = [np.ascontiguousarray(value[b].T).astype(BF16) for b in range(B)]

    in_maps = []
    for c in range(N_CORES):
        b, g = c // 4, c % 4
        cols = slice(g * EC, (g + 1) * EC)
        heads = slice(g * HPC, (g + 1) * HPC)
        bias_t = np.exp(np.ascontiguousarray(
            rel_pos_bias[b, heads].swapaxes(1, 2))).astype(BF16)
        in_maps.append({
            "qt": qT[b],
            "kt": kT[b],
            "vt": vT[b],
            "wq": (Wq[:, cols] * SCALE).astype(BF16),
            "wk": Wk[:, cols].astype(BF16),
            "wv": Wv[:, cols].astype(BF16),
            "wo": np.ascontiguousarray(Wo[cols, :]).astype(BF16),
            "bias_t": bias_t,
        })

    if PROFILE:
        _ensure_profile_hooks()
    res = run_bass_kernel_spmd(nc, in_maps, list(range(N_CORES)),
                               trace=PROFILE)
    LAST_EXEC_TIME_NS = res.exec_time_ns

    out = np.zeros((B, I, QE), dtype=np.float32)
    for c in range(N_CORES):
        out[c // 4] += np.asarray(res.results[c]["yt"]).astype(np.float32).T
    out += bo

    # generality fallbacks (never hit with all-true masks)
    if not query_mask.all() or not key_mask.reshape(B, -1).any(axis=1).all():
        for b in range(B):
            uni = value[b].mean(axis=0) @ Wv @ Wo + bo  # uniform-attention row
            if not key_mask[b].any():
                out[b, :, :] = uni
            else:
                out[b, ~query_mask[b], :] = uni
    return out


# revision 3
# speedup vs baseline: 1.0128x; 1.0128x over previous
"""Trainium2 Bass kernel for nn_CrossAttention (B=2, I=J=2048, E=1024, H=16, D=64).

Sharding: 8 cores = data parallel on batch (2) x tensor parallel on heads
(4 groups of 4 heads).  Core c handles batch c//4, heads 4*(c%4) .. 4*(c%4)+3.
Each core computes a partial output projection (its heads' slice of Wo rows);
the host sums the 4 partials per batch and adds bo.

Device-side dataflow (transposed layout; contraction dim on partitions):
  qT = Wq_g^T @ query^T          [256, 2048]   (Wq pre-scaled by D**-0.5)
  kT = Wk_g^T @ key^T            [256, 2048]
  v  = value @ Wv_g (+ones col)  [2048, 4*65]
  per head PAIR (2e, 2e+1), per i-half (1024 cols), software-pipelined jc loop:
    simT_h[j,i] = matmul(lhsT=kT[e][64h%2 slice], rhs=qT[e][same slice])
                  -- K=64 row-tiled: both heads run CONCURRENTLY in the array
    PT_h = exp(simT_h) * exp(biasT_h)   (ACT exp, DVE mult, bf16)
    o_ps_h[65, 512] += [v_h|1]^T @ PT_h  (row 64 = softmax denominator)
  normalize via transpose + reciprocal (as before), merge heads -> om
  yT = Wo_g^T' @ outT   [1024, 2048] bf16 -> DRAM
Host: out[b] = sum_g yT_g^T + bo  (f32 accumulation of bf16 partials).
"""

import os
import numpy as np
import ml_dtypes

import concourse.bass as bass
import concourse.tile as tile
from concourse import bacc, mybir
from concourse.bass_utils import run_bass_kernel_spmd
from concourse.masks import make_identity

BF16 = ml_dtypes.bfloat16
F32 = mybir.dt.float32
BF = mybir.dt.bfloat16

B, I, J = 2, 2048, 2048
E, H = 1024, 16
QE, KE = 1024, 1024
D = E // H                      # 64
SCALE = D ** -0.5
NEG = -1e20

N_CORES = 8
HPC = H // 4                    # 4 heads per core
EC = HPC * D                    # 256 E-columns per core
P = 128

# module-level switches (test.py pokes these)
PROFILE = bool(os.environ.get("KERNEL_PROFILE"))
LAST_EXEC_TIME_NS = None

_CACHED = None  # compiled Bass module
_HOOK_READY = False


def _ensure_profile_hooks():
    """Dev-only: register the NTFF profile hook that the agent image's
    antenv package lacks, and stub out the artifact upload (no bucket
    creds here).  Only used when PROFILE is on; the plain execution
    path never touches any of this."""
    global _HOOK_READY
    if _HOOK_READY:
        return
    import contextlib
    import ctypes
    import sys
    import types

    from concourse import bass_utils as bu

    bu.upload_artifacts = lambda tmpdir: "local://" + tmpdir

    try:
        from antenv.axon_hooks import get_axon_ntff_profile_hook  # noqa: F401
        _HOOK_READY = True
        return
    except ImportError:
        pass

    so_path = "/opt/axon/libaxon_pjrt.so"
    lib = ctypes.CDLL(so_path)
    assert hasattr(lib, "axon_start_nrt_profile"), "old libaxon_pjrt.so"
    lib.axon_start_nrt_profile.argtypes = [
        ctypes.POINTER(ctypes.c_int64), ctypes.c_size_t]
    lib.axon_start_nrt_profile.restype = ctypes.c_int64
    lib.axon_stop_nrt_profile.argtypes = [ctypes.c_char_p]
    lib.axon_stop_nrt_profile.restype = ctypes.c_int64

    @contextlib.contextmanager
    def _hook(output_dir, device_ids):
        import jax
        jax.devices()
        if device_ids:
            ids = (ctypes.c_int64 * len(device_ids))(*device_ids)
            rc = lib.axon_start_nrt_profile(ids, len(device_ids))
        else:
            rc = lib.axon_start_nrt_profile(None, 0)
        if rc != 0:
            raise RuntimeError(f"axon_start_nrt_profile rc={rc}")
        try:
            yield
        finally:
            n = lib.axon_stop_nrt_profile(str(output_dir).encode())
            if n < 0:
                raise RuntimeError(f"axon_stop_nrt_profile rc={n}")

    mod = types.ModuleType("antenv.axon_hooks")
    mod.get_axon_ntff_profile_hook = lambda: _hook
    mod.set_axon_ntff_profile_hook = lambda h: None
    sys.modules["antenv.axon_hooks"] = mod
    _HOOK_READY = True


def _build_program():
    nc = bacc.Bacc("TRN2", debug=False, enable_asserts=False,
                   target_bir_lowering=False, num_devices=N_CORES)

    qt_d = nc.dram_tensor("qt", [QE, I], BF, kind="ExternalInput").ap()
    kt_d = nc.dram_tensor("kt", [KE, J], BF, kind="ExternalInput").ap()
    vt_d = nc.dram_tensor("vt", [KE, J], BF, kind="ExternalInput").ap()
    wq_d = nc.dram_tensor("wq", [QE, EC], BF, kind="ExternalInput").ap()
    wk_d = nc.dram_tensor("wk", [KE, EC], BF, kind="ExternalInput").ap()
    wv_d = nc.dram_tensor("wv", [KE, EC], BF, kind="ExternalInput").ap()
    wo_d = nc.dram_tensor("wo", [EC, QE], BF, kind="ExternalInput").ap()
    bias_d = nc.dram_tensor("bias_t", [HPC, J, I], BF, kind="ExternalInput").ap()
    yt_d = nc.dram_tensor("yt", [QE, I], BF, kind="ExternalOutput").ap()

    KC = QE // P                # 8 contraction chunks for the projections
    ICN = I // 512              # 4 free-dim chunks of 512
    JC = J // P                 # 16 key chunks of 128
    VW = D + 1                  # 65: per-head v columns + ones column
    IHW = 1024                  # i-half width
    NIH = I // IHW              # 2 i-halves

    with tile.TileContext(nc) as tc:
        with (
            tc.tile_pool(name="w", bufs=1) as w_pool,
            tc.tile_pool(name="io", bufs=8) as io_pool,
            tc.tile_pool(name="persist", bufs=1) as pp,
            tc.tile_pool(name="bias", bufs=4) as bias_pool,
            tc.tile_pool(name="x", bufs=6) as x_pool,
            tc.tile_pool(name="pt", bufs=4) as pt_pool,
            tc.tile_pool(name="osb", bufs=8) as osb_pool,
            tc.tile_pool(name="rcp", bufs=4) as rcp_pool,
            tc.tile_pool(name="y", bufs=3) as y_pool,
            tc.tile_pool(name="pssim", bufs=2, space="PSUM") as ps_sim,
            tc.tile_pool(name="psot", bufs=4, space="PSUM") as ps_ot,
        ):
            # ---- weights ----
            wq_s = w_pool.tile([P, KC, EC], BF)
            wk_s = w_pool.tile([P, KC, EC], BF)
            wv_s = w_pool.tile([P, KC, EC], BF)
            wo_s = w_pool.tile([P, 2, QE], BF)
            nc.sync.dma_start(wq_s, wq_d.rearrange("(kc p) n -> p kc n", p=P))
            nc.sync.dma_start(wk_s, wk_d.rearrange("(kc p) n -> p kc n", p=P))
            nc.sync.dma_start(wv_s, wv_d.rearrange("(kc p) n -> p kc n", p=P))
            nc.sync.dma_start(wo_s, wo_d.rearrange("(ec p) n -> p ec n", p=P))
            ident = w_pool.tile([P, P], BF)
            make_identity(nc, ident)

            # ---- projections ----
            # kT_s / qT_s: [2 x (128 E-rows), N]; head h lives in tile h//2,
            # partitions 64*(h%2) .. +64.  The K=64 slices feed row-tiled
            # sim matmuls (auto tile_position from base partition).
            kT_s = [pp.tile([P, J], BF, name=f"kT{e}") for e in range(2)]
            qT_s = [pp.tile([P, I], BF, name=f"qT{e}") for e in range(2)]
            # v natural layout with a ones column per head: [J, 4*65]
            v_s = pp.tile([P, JC, HPC * VW], BF, name="v_s")
            for h in range(HPC):
                nc.gpsimd.memset(v_s[:, :, h * VW + D : h * VW + D + 1], 1.0)

            def load_chunks(src):
                ch = []
                for kc in range(KC):
                    t = io_pool.tile([P, I], BF, tag="io")
                    nc.sync.dma_start(t, src[kc * P : (kc + 1) * P, :])
                    ch.append(t)
                return ch

            # q/k projections -> packed transposed layout.  kc-outer loop
            # keeps each weight slice stationary across 4 matmuls so the
            # LDWEIGHTS pipelines under the previous matmuls.
            for src, dst, w_s in ((qt_d, qT_s, wq_s), (kt_d, kT_s, wk_s)):
                chunks = load_chunks(src)
                for e in range(2):
                    pss = [ps_ot.tile([P, 512], F32, tag="ot", name="p_ps")
                           for _ in range(ICN)]
                    for kc in range(KC):
                        for ic in range(ICN):
                            nc.tensor.matmul(
                                pss[ic],
                                lhsT=w_s[:, kc, e * P : (e + 1) * P],
                                rhs=chunks[kc][:, ic * 512 : (ic + 1) * 512],
                                start=(kc == 0), stop=(kc == KC - 1),
                            )
                    for ic in range(ICN):
                        nc.vector.tensor_copy(
                            dst[e][:, ic * 512 : (ic + 1) * 512], pss[ic])

            # v projection -> natural layout [J, EC], head-strided with ones col
            chunks = load_chunks(vt_d)
            for jc in range(JC):
                ps = ps_ot.tile([P, EC], F32, tag="ot", name="v_ps")
                for kc in range(KC):
                    nc.tensor.matmul(
                        ps,
                        lhsT=chunks[kc][:, jc * P : (jc + 1) * P],
                        rhs=wv_s[:, kc, :],
                        start=(kc == 0), stop=(kc == KC - 1),
                    )
                for h in range(HPC):
                    nc.vector.tensor_copy(
                        v_s[:, jc, h * VW : h * VW + D],
                        ps[:, h * D : (h + 1) * D],
                    )

            # out_merged: normalized attention output in natural layout
            om = pp.tile([P, I // P, EC], BF, name="om")

            # ---- attention: per-head, full-I jc units, attn lags 1 jc ----
            # Per (h, jc): one kT weight-load covers 4 sim matmuls (full I),
            # one v weight-load covers 4 attn matmuls for the previous jc,
            # so the PE has attn work while ACT/DVE chew on exp/mult.
            for h in range(HPC):
                e, psl = h // 2, slice((h % 2) * D, (h % 2) * D + D)
                o_ps = [ps_ot.tile([VW, 512], F32, tag="ot",
                                   name=f"o{h}_{c}") for c in range(ICN)]
                prev_pt = None
                for jc in range(JC + 1):
                    if jc < JC:
                        jsl = slice(jc * P, (jc + 1) * P)
                        eb = bias_pool.tile([P, I], BF, tag="bias")
                        nc.sync.dma_start(eb, bias_d[h, jsl, :])
                        pt = pt_pool.tile([P, I], BF, tag="pt")
                        for s in range(2):
                            s_ps = ps_sim.tile([P, 1024], F32, tag="sim")
                            for c in range(2):
                                i0 = s * 1024 + c * 512
                                nc.tensor.matmul(
                                    s_ps[:, c * 512 : (c + 1) * 512],
                                    lhsT=kT_s[e][psl, jsl],
                                    rhs=qT_s[e][psl, i0 : i0 + 512],
                                    start=True, stop=True,
                                )
                            x_t = x_pool.tile([P, 1024], BF, tag="x")
                            nc.scalar.activation(
                                x_t, s_ps, mybir.ActivationFunctionType.Exp)
                            nc.vector.tensor_tensor(
                                pt[:, s * 1024 : (s + 1) * 1024], x_t,
                                eb[:, s * 1024 : (s + 1) * 1024],
                                mybir.AluOpType.mult)
                    if jc >= 1:
                        pjc = jc - 1
                        for c in range(ICN):
                            nc.tensor.matmul(
                                o_ps[c],
                                lhsT=v_s[:, pjc, h * VW : (h + 1) * VW],
                                rhs=prev_pt[:, c * 512 : (c + 1) * 512],
                                start=(pjc == 0), stop=(pjc == JC - 1),
                            )
                    if jc < JC:
                        prev_pt = pt
                osbs = []
                for c in range(ICN):
                    osb = osb_pool.tile([VW, 512], BF, tag="osb",
                                        name=f"osb{h}_{c}")
                    nc.vector.tensor_copy(osb, o_ps[c])
                    osbs.append(osb)

                # normalize in natural layout: transpose [65, 128] blocks to
                # [128, 65]; recip of col 64 is then a per-partition scalar
                for ic in range(ICN):
                    osb = osbs[ic]
                    nat = ps_ot.tile([P, ICN, VW + 1], BF, tag="ot",
                                     name=f"nat{h}_{ic}")
                    for t in range(ICN):
                        nc.tensor.transpose(
                            nat[:, t, 0:VW],
                            osb[:, t * P : (t + 1) * P],
                            ident[0:VW, 0:VW],
                        )
                    for t in range(ICN):
                        rcp = rcp_pool.tile([P, 1], F32, tag="rcp")
                        nc.vector.reciprocal(rcp, nat[:, t, D : D + 1])
                        nc.vector.tensor_scalar_mul(
                            om[:, ICN * ic + t, h * D : (h + 1) * D],
                            nat[:, t, 0:D], rcp,
                        )

            # ---- transpose out_merged back to [EC, I] for the final matmul
            outT = [pp.tile([P, I], BF, name=f"outT{e}") for e in range(2)]
            for i16 in range(I // P):
                for e in range(2):
                    t2 = ps_ot.tile([P, P], BF, tag="ot", name=f"t2_{i16}_{e}")
                    nc.tensor.transpose(
                        t2, om[:, i16, e * P : (e + 1) * P], ident)
                    nc.scalar.copy(outT[e][:, i16 * P : (i16 + 1) * P], t2)

            # ---- output projection: yT = Wo_g^T-contraction over EC ----
            for oc in range(QE // P):
                pss = [ps_ot.tile([P, 512], F32, tag="ot", name="y_ps")
                       for _ in range(ICN)]
                for e in range(2):
                    for ic in range(ICN):
                        nc.tensor.matmul(
                            pss[ic],
                            lhsT=wo_s[:, e, oc * P : (oc + 1) * P],
                            rhs=outT[e][:, ic * 512 : (ic + 1) * 512],
                            start=(e == 0), stop=(e == 1),
                        )
                for ic in range(ICN):
                    y_sb = y_pool.tile([P, 512], BF, tag="y")
                    nc.vector.tensor_copy(y_sb, pss[ic])
                    nc.sync.dma_start(
                        yt_d[oc * P : (oc + 1) * P,
                             ic * 512 : (ic + 1) * 512],
                        y_sb,
                    )

    nc.compile()
    return nc


def kernel(query, key, value, query_mask, key_mask, rel_pos_bias,
           Wq, Wk, Wv, Wo, bo):
    global _CACHED, LAST_EXEC_TIME_NS
    query = np.asarray(query, dtype=np.float32)
    key = np.asarray(key, dtype=np.float32)
    value = np.asarray(value, dtype=np.float32)
    rel_pos_bias = np.asarray(rel_pos_bias, dtype=np.float32)
    query_mask = np.asarray(query_mask)
    key_mask = np.asarray(key_mask)
    Wq, Wk, Wv = (np.asarray(w, dtype=np.float32) for w in (Wq, Wk, Wv))
    Wo = np.asarray(Wo, dtype=np.float32)
    bo = np.asarray(bo, dtype=np.float32)

    if not key_mask.all():
        rel_pos_bias = rel_pos_bias + np.where(key_mask, 0.0, NEG)[:, None, None, :]

    if _CACHED is None:
        _CACHED = _build_program()
    nc = _CACHED

    # per-batch transposed activations (shared by the 4 cores of that batch)
    qT = [np.ascontiguousarray(query[b].T).astype(BF16) for b in range(B)]
    kT = [np.ascontiguousarray(key[b].T).astype(BF16) for b in range(B)]
    vT = [np.ascontiguousarray(value[b].T).astype(BF16) for b in range(B)]

    in_maps = []
    for c in range(N_CORES):
        b, g = c // 4, c % 4
        cols = slice(g * EC, (g + 1) * EC)
        heads = slice(g * HPC, (g + 1) * HPC)
        bias_t = np.exp(np.ascontiguousarray(
            rel_pos_bias[b, heads].swapaxes(1, 2))).astype(BF16)
        in_maps.append({
            "qt": qT[b],
            "kt": kT[b],
            "vt": vT[b],
            "wq": (Wq[:, cols] * SCALE).astype(BF16),
            "wk": Wk[:, cols].astype(BF16),
            "wv": Wv[:, cols].astype(BF16),
            "wo": np.ascontiguousarray(Wo[cols, :]).astype(BF16),
            "bias_t": bias_t,
        })

    if PROFILE:
        _ensure_profile_hooks()
    res = run_bass_kernel_spmd(nc, in_maps, list(range(N_CORES)),
                               trace=PROFILE)
    LAST_EXEC_TIME_NS = res.exec_time_ns

    out = np.zeros((B, I, QE), dtype=np.float32)
    for c in range(N_CORES):
        out[c // 4] += np.asarray(res.results[c]["yt"]).astype(np.float32).T
    out += bo

    # generality fallbacks (never hit with all-true masks)
    if not query_mask.all() or not key_mask.reshape(B, -1).any(axis=1).all():
        for b in range(B):
            uni = value[b].mean(axis=0) @ Wv @ Wo + bo  # uniform-attention row
            if not key_mask[b].any():
                out[b, :, :] = uni
            else:
                out[b, ~query_mask[b], :] = uni
    return out
